# revision 28
# baseline (speedup 1.0000x reference)
"""Cross-attention Trainium2 kernel (8 NeuronCores, SPMD).

Reference computation (per batch b):
    gate = sigmoid(relu(ctx @ W1 + b1) @ W2 + b2)        # [M, 1]
    ctxg = ctx * gate
    q = x @ Wq; k = ctxg @ Wk; v = ctxg @ Wv             # per head slices of 64
    out = softmax(q k^T / 8) v                           # per head
    y = concat_heads(out) @ Wo + bo                      # [N, 512]

Sharding: 8 cores = 4 batches x 2 query-halves. Each core computes the
FULL output rows for its (batch, 1024-query slice); host gather is pure
concatenation. x and context are pre-transposed on the host so the
kernel never runs a PE transpose.

Performance model this kernel is built around (measured on trn2):
  * The PE's HAM clock gate only reaches 2.4 GHz when the matmul stream
    stays in ONE array-tiling mode; any K<65 matmul switches the array
    to a row-tiled mode and the drain keeps the clock at 1.2 GHz. So
    EVERY matmul here contracts K in [65..128]: zero-padded operands
    make up the difference (KTz pair layout, padded gate weights, a
    65-row ones matmul for broadcasts).
  * The attention inner loop is software-pipelined: per context chunk,
    S-matmuls and the exp() activation for chunk j issue together with
    the PV matmuls of chunk j-1, so the Scalar (ACT) engine — the
    bottleneck at ~2.3us per chunk — never waits on the PE.
  * Softmax normalization (reciprocal + broadcast + multiply) runs
    inside the attention phase on engine slack, reusing the just-freed
    pv PSUM banks; the tail is only the output projection.
  * exp() doubles as PSUM eviction; no max-subtraction (|s| <~ 8).
  * PV lhsT is a 128-col slice [1 | V_h | junk] of Vt: out row 0 =
    softmax denominator, rows 1:65 = V^T E, rows 65:128 junk (never
    read). The ones column is FIRST because DVE reciprocal_approx_fast
    is only valid at partition 0.
"""

import os
import sys
from contextlib import ExitStack

import numpy as np

if "/opt/trn_rl_repo" not in sys.path:
    sys.path.insert(0, "/opt/trn_rl_repo")

import concourse.bass as bass
import concourse.mybir as mybir
import concourse.tile as tile
from concourse import bacc
from concourse.bass_utils import run_bass_kernel_spmd

F32 = mybir.dt.float32
F32R = mybir.dt.float32r
EXPF = mybir.ActivationFunctionType.Exp
RELUF = mybir.ActivationFunctionType.Relu
SIGMF = mybir.ActivationFunctionType.Sigmoid

H = 8          # heads
DH = 64        # dim per head
QD = 512       # query feature dim
CD = 64        # context feature dim
GH = 32        # gate hidden
INNER = H * DH # 512
SCALE = DH ** -0.5
VW = DH + 1    # per-head Vt block width (ones col + V)


def _r(ap):
    return ap.bitcast(F32R)


def build_core_kernel(nc, NQ=1024, M=2048):
    """Emit the per-core kernel. NQ = queries on this core, M = ctx length."""
    P = 128
    NJC = M // P          # ctx 128-chunks
    NG4 = M // 512        # ctx 512-chunks
    NQC = max(NQ // 512, 1)  # query 512-chunks
    QCW = min(512, NQ)    # query chunk width
    NKC = QD // P         # 4 qdim 128-chunks

    xt_d = nc.dram_tensor("xt_in", [QD, NQ], F32, kind="ExternalInput").ap()
    ct_d = nc.dram_tensor("ctxt_in", [CD, M], F32, kind="ExternalInput").ap()
    wq_d = nc.dram_tensor("wq_in", [QD, INNER], F32, kind="ExternalInput").ap()
    wk_d = nc.dram_tensor("wk_in", [CD, INNER], F32, kind="ExternalInput").ap()
    wv_d = nc.dram_tensor("wv_in", [CD, INNER], F32, kind="ExternalInput").ap()
    wo_d = nc.dram_tensor("wo_in", [INNER, QD], F32, kind="ExternalInput").ap()
    w1_d = nc.dram_tensor("w1_in", [CD, GH], F32, kind="ExternalInput").ap()
    w2_d = nc.dram_tensor("w2_in", [GH, 1], F32, kind="ExternalInput").ap()
    b1_d = nc.dram_tensor("b1_in", [GH, 1], F32, kind="ExternalInput").ap()
    b2_d = nc.dram_tensor("b2_in", [1, 1], F32, kind="ExternalInput").ap()
    bo_d = nc.dram_tensor("bo_in", [1, QD], F32, kind="ExternalInput").ap()
    y_d = nc.dram_tensor("y_out", [NQ, QD], F32, kind="ExternalOutput").ap()

    with TileCtx(nc) as tc, ExitStack() as ctx, \
            nc.allow_low_precision(reason="float32r rounding for PE operands"):
        const = ctx.enter_context(tc.tile_pool(name="const", bufs=1))
        persist = ctx.enter_context(tc.tile_pool(name="persist", bufs=1))
        vstack = ExitStack()
        vpool = vstack.enter_context(tc.tile_pool(name="vpool", bufs=1, side="right"))
        xpool = vstack.enter_context(tc.tile_pool(name="xpool", bufs=1, side="right"))
        early = ExitStack()
        cpool = early.enter_context(tc.tile_pool(name="cpool", bufs=1))
        psum_pj = early.enter_context(tc.tile_pool(name="psum_pj", bufs=2, space="PSUM"))

        dma = nc.sync.dma_start
        dma2 = nc.scalar.dma_start

        # ---- constants / weights (zero-padded to K=128 contractions) ----
        ones_f = const.tile([1, P], F32, tag="ones_f", name="ones_f")
        nc.vector.memset(ones_f[:], 1.0)
        # Lones65: row 0 = ones, rows 1:65 = 0. As lhsT with a [65, N] rhs it
        # broadcasts the rhs's row 0 to 65 output rows while contracting
        # K=65 (which rounds to the 128x128 PE mode — no mode switch).
        lones = const.tile([DH + 1, DH + 1], F32R, tag="lones", name="lones")
        nc.vector.memset(lones[:].bitcast(F32), 0.0)
        nc.vector.memset(lones[0:1, :].bitcast(F32), 1.0)

        wk_sb = const.tile([P, INNER], F32R, tag="wk", name="wk")
        dma(wk_sb[0:CD, :], wk_d[:, :].bitcast(F32R))
        nc.gpsimd.memset(wk_sb[CD:P, :].bitcast(F32), 0.0)
        wv_sb = const.tile([P, INNER], F32R, tag="wv", name="wv")
        dma(wv_sb[0:CD, :], wv_d[:, :].bitcast(F32R))
        nc.gpsimd.memset(wv_sb[CD:P, :].bitcast(F32), 0.0)
        w1_sb = const.tile([P, GH], F32R, tag="w1", name="w1")
        dma(w1_sb[0:CD, :], w1_d[:, :].bitcast(F32R))
        nc.vector.memset(w1_sb[CD:P, :].bitcast(F32), 0.0)
        w2_sb = const.tile([P, 1], F32R, tag="w2", name="w2")
        dma(w2_sb[0:GH, :], w2_d[:, :].bitcast(F32R))
        nc.vector.memset(w2_sb[GH:2 * GH, :].bitcast(F32), 0.0)
        nc.vector.memset(w2_sb[2 * GH:P, :].bitcast(F32), 0.0)
        b1_sb = const.tile([GH, 1], F32, tag="b1", name="b1")
        dma(b1_sb[:], b1_d[:, :])
        b2_sb = const.tile([1, 1], F32, tag="b2", name="b2")
        dma(b2_sb[:], b2_d[:, :])

        # x^T and Wq ride the Activation-engine HWDGE queue so they overlap
        # the ctx-chain DMAs on the SP queue.
        wq_sb = [xpool.tile([P, INNER], F32R, tag=f"wq{k}", name=f"wq{k}") for k in range(NKC)]
        for k in range(NKC):
            dma2(wq_sb[k][:], wq_d[k * P:(k + 1) * P, :].bitcast(F32R))
        xT = [xpool.tile([P, NQ], F32R, tag=f"xT{k}", name=f"xT{k}") for k in range(NKC)]
        for k in range(NKC):
            dma2(xT[k][:], xt_d[k * P:(k + 1) * P, :].bitcast(F32R))

        # ---- context (pre-transposed on host), bottom rows zeroed ----
        ctxT = cpool.tile([P, M], F32R, tag="ctxT", name="ctxT")
        dma(ctxT[0:CD, :], ct_d[:, :].bitcast(F32R))
        nc.gpsimd.memset(ctxT[CD:P, :].bitcast(F32), 0.0)

        # ---- gate + gated context: ctxgT [128, M] (bottom zero) ----
        ctxgT = cpool.tile([P, M], F32R, tag="ctxgT", name="ctxgT")
        nc.gpsimd.memset(ctxgT[CD:P, :].bitcast(F32), 0.0)
        h1z = cpool.tile([P, 512], F32R, tag="h1z", name="h1z")
        nc.vector.memset(h1z[GH:2 * GH, :].bitcast(F32), 0.0)
        nc.vector.memset(h1z[2 * GH:P, :].bitcast(F32), 0.0)
        gtz = cpool.tile([DH + 1, 512], F32R, tag="gtz", name="gtz")
        nc.vector.memset(gtz[:].bitcast(F32), 0.0)
        for g in range(NG4):
            sl = slice(g * 512, (g + 1) * 512)
            pp = psum_pj.tile([P, 512], F32, tag="pj", name="pj")
            nc.tensor.matmul(pp[0:GH, :], _r(w1_sb[:]), _r(ctxT[:, sl]),
                             start=True, stop=True)
            nc.scalar.activation(h1z[0:GH, :], pp[0:GH, :], RELUF, bias=b1_sb[:])
            pp2 = psum_pj.tile([P, 512], F32, tag="pj", name="pj")
            nc.tensor.matmul(pp2[0:1, :], _r(w2_sb[:]), _r(h1z[:]),
                             start=True, stop=True)
            nc.scalar.activation(gtz[0:1, :], pp2[0:1, :], SIGMF, bias=b2_sb[:])
            ppb = psum_pj.tile([P, 512], F32, tag="pj", name="pj")
            nc.tensor.matmul(ppb[0:CD, :], _r(lones[:, 0:CD]), _r(gtz[:]),
                             start=True, stop=True)
            nc.vector.tensor_mul(ctxgT[0:CD, sl], ctxT[0:CD, sl], ppb[0:CD, :])

        # ---- K^T, zero-padded pair layout: KTz[h] [128, M] ----
        # even head: data rows 0:64, zeros 64:128; odd head: the reverse —
        # matching QT's pair layout so S contracts K=128 against zeros.
        KTz = [persist.tile([P, M], F32R, tag=f"ktz{h}", name=f"ktz{h}") for h in range(H)]
        for h in range(H):
            z = slice(DH, P) if h % 2 == 0 else slice(0, DH)
            nc.gpsimd.memset(KTz[h][z, :].bitcast(F32), 0.0)
        for pr in range(H // 2):
            for g in range(NG4):
                sl = slice(g * 512, (g + 1) * 512)
                pp = psum_pj.tile([P, 512], F32, tag="pj", name="pj")
                nc.tensor.matmul(pp[:], _r(wk_sb[:, pr * P:(pr + 1) * P]),
                                 _r(ctxgT[:, sl]), start=True, stop=True)
                nc.vector.tensor_copy(KTz[2 * pr][0:DH, sl], pp[0:DH, :])
                nc.scalar.copy(KTz[2 * pr + 1][DH:P, sl], pp[DH:P, :])

        # ---- Q^T pair 0 first so attention can start early ----
        QT = [persist.tile([P, NQ], F32R, tag=f"qt{pr}", name=f"qt{pr}") for pr in range(H // 2)]

        def gen_qt(pr):
            for qc in range(NQC):
                sl = slice(qc * QCW, (qc + 1) * QCW)
                if pr == 0:
                    pp = psum_pj.tile([P, 512], F32, tag="pj", name="pj")
                else:
                    pp = psum_st.tile([P, NQC * 512], F32, tag=f"st{qc}",
                                      name="qtp")
                for k in range(NKC):
                    nc.tensor.matmul(pp[:, 0:QCW],
                                     _r(wq_sb[k][:, pr * P:(pr + 1) * P]),
                                     _r(xT[k][:, sl]),
                                     start=(k == 0), stop=(k == NKC - 1))
                # ACT is saturated with exp() during attention — only use it
                # for QT eviction in the early phase (pr == 0).
                cp = nc.scalar.copy if (pr == 0 and qc % 2 == 1) \
                    else nc.vector.tensor_copy
                cp(QT[pr][:, sl], pp[:, 0:QCW])

        gen_qt(0)

        # ---- V natural, interleaved [1 | V_h] blocks of 65 + 63-col pad so
        # every head has a 128-col stationary slice: Vt[t] [128, 583] ----
        VTW = H * VW + (P - VW)
        Vt = [vpool.tile([P, VTW], F32R, tag=f"v{t}", name=f"v{t}") for t in range(NJC)]
        for t in range(NJC):
            nc.gpsimd.memset(Vt[t][:].bitcast(F32), 0.0)
            vv = Vt[t][:, 0:H * VW].rearrange("p (h c) -> p h c", c=VW)
            nc.gpsimd.memset(vv[:, :, 0:1].bitcast(F32), 1.0)
            pp = psum_pj.tile([P, 512], F32, tag="pj", name="pj")
            nc.tensor.matmul(pp[:], _r(ctxgT[:, t * P:(t + 1) * P]), _r(wv_sb[:]),
                             start=True, stop=True)
            cp = nc.vector.tensor_copy if t % 2 == 0 else nc.scalar.copy
            cp(vv[:, :, 1:VW], pp[:].rearrange("p (h c) -> p h c", c=DH))

        early.close()
        attn = ExitStack()
        psum_st = attn.enter_context(tc.tile_pool(name="psum_st", bufs=1, space="PSUM"))
        psum_pv = attn.enter_context(tc.tile_pool(name="psum_pv", bufs=1, space="PSUM"))
        epool = ctx.enter_context(tc.tile_pool(name="epool", bufs=2))
        rpool = ctx.enter_context(tc.tile_pool(name="rpool", bufs=2))

        # ---- attention, software-pipelined per context chunk ----
        # Per jc: S(h0), EXP(h0), PV(h0, jc-1), S(h1), EXP(h1), PV(h1, jc-1)
        # so the ACT engine (the ~2.3us/jc bottleneck) runs back to back.
        OT = [persist.tile([DH + 1, NQ], F32R, tag=f"ot{h}", name=f"ot{h}") for h in range(H)]
        for pr in range(H // 2):
            pv = {(hp, qc): psum_pv.tile([P, 512], F32,
                                         tag=f"pv{hp}{qc}", name=f"pv{hp}{qc}")
                  for hp in range(2) for qc in range(NQC)}
            et_prev = [None, None]
            for jc in range(NJC):
                et_cur = [None, None]
                for hp in range(2):
                    st = psum_st.tile([P, NQC * 512], F32, tag=f"st{hp}",
                                      name=f"st{hp}")
                    for qc in range(NQC):
                        nc.tensor.matmul(
                            st[:, qc * 512:qc * 512 + QCW],
                            _r(KTz[2 * pr + hp][:, jc * P:(jc + 1) * P]),
                            _r(QT[pr][:, qc * QCW:(qc + 1) * QCW]),
                            start=True, stop=True)
                    et = epool.tile([P, NQC * 512], F32R, tag=f"e{hp}",
                                    name=f"et{hp}")
                    nc.scalar.activation(et[:], st[:], EXPF, scale=SCALE)
                    et_cur[hp] = et
                    if jc > 0:
                        h = 2 * pr + hp
                        for qc in range(NQC):
                            nc.tensor.matmul(
                                pv[(hp, qc)][:, 0:QCW],
                                _r(Vt[jc - 1][:, h * VW:h * VW + P]),
                                _r(et_prev[hp][:, qc * 512:qc * 512 + QCW]),
                                start=(jc == 1), stop=False)
                et_prev = et_cur
            for hp in range(2):
                h = 2 * pr + hp
                for qc in range(NQC):
                    nc.tensor.matmul(
                        pv[(hp, qc)][:, 0:QCW],
                        _r(Vt[NJC - 1][:, h * VW:h * VW + P]),
                        _r(et_prev[hp][:, qc * 512:qc * 512 + QCW]),
                        start=False, stop=True)
            # Next pair's QT projection rides the boundary first so its
            # eviction copies sit at the head of the DVE queue — the next
            # pair's S matmuls wait only on them, not on the normalize work.
            if pr + 1 < H // 2:
                gen_qt(pr + 1)
            # Evict rows 0:65 (row 0 = denominator) and replace the
            # denominator with its reciprocal; the broadcast/multiply runs in
            # the tail so no PE instruction here depends on the DVE chain.
            for hp in range(2):
                h = 2 * pr + hp
                for qc in range(NQC):
                    sl = slice(qc * QCW, (qc + 1) * QCW)
                    nc.vector.tensor_copy(OT[h][:, sl],
                                          pv[(hp, qc)][0:DH + 1, 0:QCW])
                    rcf = rpool.tile([1, 512], F32, tag="rcf", name="rcf")
                    nc.vector.reciprocal_approx_fast(
                        rcf[:, 0:QCW], OT[h][0:1, sl].bitcast(F32))
                    nc.vector.tensor_copy(OT[h][0:1, sl], rcf[:, 0:QCW])
        attn.close()
        vstack.close()
        psum_tl = ctx.enter_context(tc.tile_pool(name="psum_tl", bufs=3, space="PSUM"))
        wopool = ctx.enter_context(tc.tile_pool(name="wopool", bufs=1))
        # Wo per head, shifted one row down; row 0 holds bo (head 0) / 0.
        wo_sb = []
        for h in range(H):
            t = wopool.tile([DH + 1, QD], F32R, tag=f"wo{h}", name=f"wo{h}")
            dma(t[1:DH + 1, :], wo_d[h * DH:(h + 1) * DH, :].bitcast(F32R))
            if h == 0:
                dma(t[0:1, :], bo_d[:, :].bitcast(F32R))
            else:
                nc.gpsimd.memset(t[0:1, :].bitcast(F32), 0.0)
            wo_sb.append(t)

        # ---- normalize (broadcast 1/denom from row 0, multiply, pin row 0
        # to 1.0 for the bias trick) + output projection ----
        for qc in range(NQC):
            sl = slice(qc * QCW, (qc + 1) * QCW)
            for h in range(H):
                rb = psum_tl.tile([DH + 1, 512], F32, tag="rb", name="rb")
                nc.tensor.matmul(rb[:, 0:QCW], _r(lones[:]),
                                 _r(OT[h][:, sl]), start=True, stop=True)
                nc.vector.tensor_mul(OT[h][:, sl], OT[h][:, sl].bitcast(F32),
                                     rb[:, 0:QCW])
                nc.gpsimd.memset(OT[h][0:1, sl].bitcast(F32), 1.0)
            for q8 in range(qc * QCW // P, (qc + 1) * QCW // P):
                po = psum_tl.tile([P, 512], F32, tag="po", name="po")
                for h in range(H):
                    nc.tensor.matmul(po[:],
                                     _r(OT[h][:, q8 * P:(q8 + 1) * P]),
                                     _r(wo_sb[h][:, :]),
                                     start=(h == 0), stop=(h == H - 1))
                cp = nc.vector.tensor_copy if q8 % 2 == 0 else nc.scalar.copy
                ost = rpool.tile([P, 512], F32, tag="ost", name="ost")
                cp(ost[:], po[:])
                dma(y_d[q8 * P:(q8 + 1) * P, :], ost[:])

    return nc


def TileCtx(nc):
    return tile.TileContext(nc)


_NC_CACHE = {}


def _get_compiled(NQ=1024, M=2048):
    key = (NQ, M)
    if key not in _NC_CACHE:
        nc = bacc.Bacc("TRN2", target_bir_lowering=False, debug=False)
        build_core_kernel(nc, NQ=NQ, M=M)
        nc.compile()
        _NC_CACHE[key] = nc
    return _NC_CACHE[key]


def _make_in_maps(inputs):
    x = np.ascontiguousarray(np.asarray(inputs["x"], dtype=np.float32))
    context = np.ascontiguousarray(np.asarray(inputs["context"], dtype=np.float32))
    B, N, _ = x.shape
    NQ = N // 2
    common = {
        "wq_in": np.asarray(inputs["Wq"], np.float32),
        "wk_in": np.asarray(inputs["Wk"], np.float32),
        "wv_in": np.asarray(inputs["Wv"], np.float32),
        "wo_in": np.asarray(inputs["Wo"], np.float32),
        "w1_in": np.asarray(inputs["W1"], np.float32),
        "w2_in": np.asarray(inputs["W2"], np.float32).reshape(GH, 1),
        "b1_in": np.asarray(inputs["b1"], np.float32).reshape(GH, 1),
        "b2_in": np.asarray(inputs["b2"], np.float32).reshape(1, 1),
        "bo_in": np.asarray(inputs["bo"], np.float32).reshape(1, QD),
    }
    in_maps = []
    for c in range(8):
        b, qh = c // 2, c % 2
        m = dict(common)
        m["xt_in"] = np.ascontiguousarray(x[b, qh * NQ:(qh + 1) * NQ, :].T)
        m["ctxt_in"] = np.ascontiguousarray(context[b].T)
        in_maps.append(m)
    return in_maps


def kernel(x, context, Wq, Wk, Wv, W1, b1, W2, b2, Wo, bo):
    x = np.ascontiguousarray(np.asarray(x, dtype=np.float32))
    context = np.ascontiguousarray(np.asarray(context, dtype=np.float32))
    B, N, _ = x.shape
    NQ = N // 2
    M = context.shape[1]
    nc = _get_compiled(NQ=NQ, M=M)
    in_maps = _make_in_maps(dict(
        x=x, context=context, Wq=Wq, Wk=Wk, Wv=Wv, W1=W1, b1=b1, W2=W2,
        b2=b2, Wo=Wo, bo=bo))

    res = run_bass_kernel_spmd(nc, in_maps, list(range(8))).results
    out = np.empty((B, N, QD), dtype=np.float32)
    for c in range(8):
        b, qh = c // 2, c % 2
        out[b, qh * NQ:(qh + 1) * NQ, :] = res[c]["y_out"]
    return out


# revision 29
# speedup vs baseline: 1.1248x; 1.1248x over previous
"""Cross-attention Trainium2 kernel (8 NeuronCores, SPMD).

Reference computation (per batch b):
    gate = sigmoid(relu(ctx @ W1 + b1) @ W2 + b2)        # [M, 1]
    ctxg = ctx * gate
    q = x @ Wq; k = ctxg @ Wk; v = ctxg @ Wv             # per head slices of 64
    out = softmax(q k^T / 8) v                           # per head
    y = concat_heads(out) @ Wo + bo                      # [N, 512]

Sharding: 8 cores = 4 batches x 2 query-halves. Each core computes the
FULL output rows for its (batch, 1024-query slice); host gather is pure
concatenation. x and context are pre-transposed on the host so the
kernel never runs a PE transpose.

Performance model this kernel is built around (measured on trn2):
  * The PE's HAM clock gate only reaches 2.4 GHz when the matmul stream
    stays in ONE array-tiling mode; any K<65 matmul switches the array
    to a row-tiled mode and the drain keeps the clock at 1.2 GHz. So
    EVERY matmul here contracts K in [65..128]: zero-padded operands
    make up the difference (KTz pair layout, padded gate weights, a
    65-row ones matmul for broadcasts).
  * The attention inner loop is software-pipelined: per context chunk,
    S-matmuls and the exp() activation for chunk j issue together with
    the PV matmuls of chunk j-1, so the Scalar (ACT) engine — the
    bottleneck at ~2.3us per chunk — never waits on the PE.
  * Softmax normalization (reciprocal + broadcast + multiply) runs
    inside the attention phase on engine slack, reusing the just-freed
    pv PSUM banks; the tail is only the output projection.
  * exp() doubles as PSUM eviction; no max-subtraction (|s| <~ 8).
  * PV lhsT is a 128-col slice [1 | V_h | junk] of Vt: out row 0 =
    softmax denominator, rows 1:65 = V^T E, rows 65:128 junk (never
    read). The ones column is FIRST because DVE reciprocal_approx_fast
    is only valid at partition 0.
"""

import os
import sys
from contextlib import ExitStack

import numpy as np

if "/opt/trn_rl_repo" not in sys.path:
    sys.path.insert(0, "/opt/trn_rl_repo")

import concourse.bass as bass
import concourse.mybir as mybir
import concourse.tile as tile
from concourse import bacc
from concourse.bass_utils import run_bass_kernel_spmd

F32 = mybir.dt.float32
F32R = mybir.dt.float32r
EXPF = mybir.ActivationFunctionType.Exp
RELUF = mybir.ActivationFunctionType.Relu
SIGMF = mybir.ActivationFunctionType.Sigmoid

H = 8          # heads
DH = 64        # dim per head
QD = 512       # query feature dim
CD = 64        # context feature dim
GH = 32        # gate hidden
INNER = H * DH # 512
SCALE = DH ** -0.5
VW = DH + 1    # per-head Vt block width (ones col + V)


def _r(ap):
    return ap.bitcast(F32R)


def build_core_kernel(nc, NQ=1024, M=2048):
    """Emit the per-core kernel. NQ = queries on this core, M = ctx length."""
    P = 128
    NJC = M // P          # ctx 128-chunks
    NG4 = M // 512        # ctx 512-chunks
    NQC = max(NQ // 512, 1)  # query 512-chunks
    QCW = min(512, NQ)    # query chunk width
    NKC = QD // P         # 4 qdim 128-chunks

    xt_d = nc.dram_tensor("xt_in", [QD, NQ], F32, kind="ExternalInput").ap()
    ct_d = nc.dram_tensor("ctxt_in", [CD, M], F32, kind="ExternalInput").ap()
    wq_d = nc.dram_tensor("wq_in", [QD, INNER], F32, kind="ExternalInput").ap()
    wk_d = nc.dram_tensor("wk_in", [CD, INNER], F32, kind="ExternalInput").ap()
    wv_d = nc.dram_tensor("wv_in", [CD, INNER], F32, kind="ExternalInput").ap()
    wo_d = nc.dram_tensor("wo_in", [INNER, QD], F32, kind="ExternalInput").ap()
    w1_d = nc.dram_tensor("w1_in", [CD, GH], F32, kind="ExternalInput").ap()
    w2_d = nc.dram_tensor("w2_in", [GH, 1], F32, kind="ExternalInput").ap()
    b1_d = nc.dram_tensor("b1_in", [GH, 1], F32, kind="ExternalInput").ap()
    b2_d = nc.dram_tensor("b2_in", [1, 1], F32, kind="ExternalInput").ap()
    bo_d = nc.dram_tensor("bo_in", [1, QD], F32, kind="ExternalInput").ap()
    y_d = nc.dram_tensor("y_out", [NQ, QD], F32, kind="ExternalOutput").ap()

    with TileCtx(nc) as tc, ExitStack() as ctx, \
            nc.allow_low_precision(reason="float32r rounding for PE operands"):
        const = ctx.enter_context(tc.tile_pool(name="const", bufs=1))
        persist = ctx.enter_context(tc.tile_pool(name="persist", bufs=1))
        vstack = ExitStack()
        vpool = vstack.enter_context(tc.tile_pool(name="vpool", bufs=1, side="right"))
        xpool = vstack.enter_context(tc.tile_pool(name="xpool", bufs=1, side="right"))
        early = ExitStack()
        cpool = early.enter_context(tc.tile_pool(name="cpool", bufs=1))
        psum_pj = early.enter_context(tc.tile_pool(name="psum_pj", bufs=2, space="PSUM"))

        dma = nc.sync.dma_start
        dma2 = nc.scalar.dma_start

        # ---- constants / weights (zero-padded to K=128 contractions) ----
        ones_f = const.tile([1, P], F32, tag="ones_f", name="ones_f")
        nc.vector.memset(ones_f[:], 1.0)
        # Lones65: row 0 = ones, rows 1:65 = 0. As lhsT with a [65, N] rhs it
        # broadcasts the rhs's row 0 to 65 output rows while contracting
        # K=65 (which rounds to the 128x128 PE mode — no mode switch).
        lones = const.tile([DH + 1, DH + 1], F32R, tag="lones", name="lones")
        nc.vector.memset(lones[:].bitcast(F32), 0.0)
        nc.vector.memset(lones[0:1, :].bitcast(F32), 1.0)

        wk_sb = const.tile([P, INNER], F32R, tag="wk", name="wk")
        dma(wk_sb[0:CD, :], wk_d[:, :].bitcast(F32R))
        nc.gpsimd.memset(wk_sb[CD:P, :].bitcast(F32), 0.0)
        wv_sb = const.tile([P, INNER], F32R, tag="wv", name="wv")
        dma(wv_sb[0:CD, :], wv_d[:, :].bitcast(F32R))
        nc.gpsimd.memset(wv_sb[CD:P, :].bitcast(F32), 0.0)
        w1_sb = const.tile([P, GH], F32R, tag="w1", name="w1")
        dma(w1_sb[0:CD, :], w1_d[:, :].bitcast(F32R))
        nc.vector.memset(w1_sb[CD:P, :].bitcast(F32), 0.0)
        w2_sb = const.tile([P, 1], F32R, tag="w2", name="w2")
        dma(w2_sb[0:GH, :], w2_d[:, :].bitcast(F32R))
        nc.vector.memset(w2_sb[GH:2 * GH, :].bitcast(F32), 0.0)
        nc.vector.memset(w2_sb[2 * GH:P, :].bitcast(F32), 0.0)
        b1_sb = const.tile([GH, 1], F32, tag="b1", name="b1")
        dma(b1_sb[:], b1_d[:, :])
        b2_sb = const.tile([1, 1], F32, tag="b2", name="b2")
        dma(b2_sb[:], b2_d[:, :])

        # x^T and Wq ride the Activation-engine HWDGE queue so they overlap
        # the ctx-chain DMAs on the SP queue.
        wq_sb = [xpool.tile([P, INNER], F32R, tag=f"wq{k}", name=f"wq{k}") for k in range(NKC)]
        for k in range(NKC):
            dma2(wq_sb[k][:], wq_d[k * P:(k + 1) * P, :].bitcast(F32R))
        xT = [xpool.tile([P, NQ], F32R, tag=f"xT{k}", name=f"xT{k}") for k in range(NKC)]
        for k in range(NKC):
            dma2(xT[k][:], xt_d[k * P:(k + 1) * P, :].bitcast(F32R))

        # ---- context (pre-transposed on host), bottom rows zeroed ----
        ctxT = cpool.tile([P, M], F32R, tag="ctxT", name="ctxT")
        dma(ctxT[0:CD, :], ct_d[:, :].bitcast(F32R))
        nc.gpsimd.memset(ctxT[CD:P, :].bitcast(F32), 0.0)

        # ---- gate + gated context: ctxgT [128, M] (bottom zero) ----
        ctxgT = cpool.tile([P, M], F32R, tag="ctxgT", name="ctxgT")
        nc.gpsimd.memset(ctxgT[CD:P, :].bitcast(F32), 0.0)
        h1z = cpool.tile([P, 512], F32R, tag="h1z", name="h1z")
        nc.vector.memset(h1z[GH:2 * GH, :].bitcast(F32), 0.0)
        nc.vector.memset(h1z[2 * GH:P, :].bitcast(F32), 0.0)
        gtz = cpool.tile([DH + 1, 512], F32R, tag="gtz", name="gtz")
        nc.vector.memset(gtz[:].bitcast(F32), 0.0)
        for g in range(NG4):
            sl = slice(g * 512, (g + 1) * 512)
            pp = psum_pj.tile([P, 512], F32, tag="pj", name="pj")
            nc.tensor.matmul(pp[0:GH, :], _r(w1_sb[:]), _r(ctxT[:, sl]),
                             start=True, stop=True)
            nc.scalar.activation(h1z[0:GH, :], pp[0:GH, :], RELUF, bias=b1_sb[:])
            pp2 = psum_pj.tile([P, 512], F32, tag="pj", name="pj")
            nc.tensor.matmul(pp2[0:1, :], _r(w2_sb[:]), _r(h1z[:]),
                             start=True, stop=True)
            nc.scalar.activation(gtz[0:1, :], pp2[0:1, :], SIGMF, bias=b2_sb[:])
            ppb = psum_pj.tile([P, 512], F32, tag="pj", name="pj")
            nc.tensor.matmul(ppb[0:CD, :], _r(lones[:, 0:CD]), _r(gtz[:]),
                             start=True, stop=True)
            nc.vector.tensor_mul(ctxgT[0:CD, sl], ctxT[0:CD, sl], ppb[0:CD, :])

        # ---- K^T, zero-padded pair layout: KTz[h] [128, M] ----
        # even head: data rows 0:64, zeros 64:128; odd head: the reverse —
        # matching QT's pair layout so S contracts K=128 against zeros.
        KTz = [persist.tile([P, M], F32R, tag=f"ktz{h}", name=f"ktz{h}") for h in range(H)]
        for h in range(H):
            z = slice(DH, P) if h % 2 == 0 else slice(0, DH)
            nc.gpsimd.memset(KTz[h][z, :].bitcast(F32), 0.0)
        for pr in range(H // 2):
            for g in range(NG4):
                sl = slice(g * 512, (g + 1) * 512)
                pp = psum_pj.tile([P, 512], F32, tag="pj", name="pj")
                nc.tensor.matmul(pp[:], _r(wk_sb[:, pr * P:(pr + 1) * P]),
                                 _r(ctxgT[:, sl]), start=True, stop=True)
                nc.vector.tensor_copy(KTz[2 * pr][0:DH, sl], pp[0:DH, :])
                nc.scalar.copy(KTz[2 * pr + 1][DH:P, sl], pp[DH:P, :])

        # ---- Q^T pair 0 first so attention can start early ----
        QT = [persist.tile([P, NQ], F32R, tag=f"qt{pr}", name=f"qt{pr}") for pr in range(H // 2)]

        def gen_qt(pr):
            for qc in range(NQC):
                sl = slice(qc * QCW, (qc + 1) * QCW)
                if pr == 0:
                    pp = psum_pj.tile([P, 512], F32, tag="pj", name="pj")
                else:
                    pp = psum_st.tile([P, NQC * 512], F32, tag=f"st{qc}",
                                      name="qtp")
                for k in range(NKC):
                    nc.tensor.matmul(pp[:, 0:QCW],
                                     _r(wq_sb[k][:, pr * P:(pr + 1) * P]),
                                     _r(xT[k][:, sl]),
                                     start=(k == 0), stop=(k == NKC - 1))
                # ACT is saturated with exp() during attention — only use it
                # for QT eviction in the early phase (pr == 0).
                cp = nc.scalar.copy if (pr == 0 and qc % 2 == 1) \
                    else nc.vector.tensor_copy
                cp(QT[pr][:, sl], pp[:, 0:QCW])

        gen_qt(0)

        # ---- V natural, interleaved [1 | V_h] blocks of 65 + 63-col pad so
        # every head has a 128-col stationary slice: Vt[t] [128, 583] ----
        VTW = H * VW + (P - VW)
        Vt = [vpool.tile([P, VTW], F32R, tag=f"v{t}", name=f"v{t}") for t in range(NJC)]
        for t in range(NJC):
            nc.gpsimd.memset(Vt[t][:].bitcast(F32), 0.0)
            vv = Vt[t][:, 0:H * VW].rearrange("p (h c) -> p h c", c=VW)
            nc.gpsimd.memset(vv[:, :, 0:1].bitcast(F32), 1.0)
            pp = psum_pj.tile([P, 512], F32, tag="pj", name="pj")
            nc.tensor.matmul(pp[:], _r(ctxgT[:, t * P:(t + 1) * P]), _r(wv_sb[:]),
                             start=True, stop=True)
            cp = nc.vector.tensor_copy if t % 2 == 0 else nc.scalar.copy
            cp(vv[:, :, 1:VW], pp[:].rearrange("p (h c) -> p h c", c=DH))

        early.close()
        attn = ExitStack()
        psum_st = attn.enter_context(tc.tile_pool(name="psum_st", bufs=1, space="PSUM"))
        psum_pv = attn.enter_context(tc.tile_pool(name="psum_pv", bufs=1, space="PSUM"))
        epool = ctx.enter_context(tc.tile_pool(name="epool", bufs=2))
        rpool = ctx.enter_context(tc.tile_pool(name="rpool", bufs=2))

        # ---- attention, software-pipelined per context chunk ----
        # Per jc: S(h0), EXP(h0), PV(h0, jc-1), S(h1), EXP(h1), PV(h1, jc-1)
        # so the ACT engine (the ~2.3us/jc bottleneck) runs back to back.
        OT = [persist.tile([DH + 1, NQ], F32R, tag=f"ot{h}", name=f"ot{h}") for h in range(H)]
        for pr in range(H // 2):
            if pr > 0:
                gen_qt(pr)  # rides PE slack inside the previous pairs
            pv = {(hp, qc): psum_pv.tile([P, 512], F32,
                                         tag=f"pv{hp}{qc}", name=f"pv{hp}{qc}")
                  for hp in range(2) for qc in range(NQC)}
            et_prev = [None, None]
            for jc in range(NJC):
                et_cur = [None, None]
                for hp in range(2):
                    st = psum_st.tile([P, NQC * 512], F32, tag=f"st{hp}",
                                      name=f"st{hp}")
                    for qc in range(NQC):
                        nc.tensor.matmul(
                            st[:, qc * 512:qc * 512 + QCW],
                            _r(KTz[2 * pr + hp][:, jc * P:(jc + 1) * P]),
                            _r(QT[pr][:, qc * QCW:(qc + 1) * QCW]),
                            start=True, stop=True)
                    et = epool.tile([P, NQC * 512], F32R, tag=f"e{hp}",
                                    name=f"et{hp}")
                    nc.scalar.activation(et[:], st[:], EXPF, scale=SCALE)
                    et_cur[hp] = et
                    if jc > 0:
                        h = 2 * pr + hp
                        for qc in range(NQC):
                            nc.tensor.matmul(
                                pv[(hp, qc)][:, 0:QCW],
                                _r(Vt[jc - 1][:, h * VW:h * VW + P]),
                                _r(et_prev[hp][:, qc * 512:qc * 512 + QCW]),
                                start=(jc == 1), stop=False)
                et_prev = et_cur
            for hp in range(2):
                h = 2 * pr + hp
                for qc in range(NQC):
                    nc.tensor.matmul(
                        pv[(hp, qc)][:, 0:QCW],
                        _r(Vt[NJC - 1][:, h * VW:h * VW + P]),
                        _r(et_prev[hp][:, qc * 512:qc * 512 + QCW]),
                        start=False, stop=True)
            # Evict rows 0:65 (row 0 = denominator) and normalize in place on
            # engine slack: reciprocal at partition 0, broadcast via the
            # 65-row ones matmul into the just-freed pv bank, multiply, then
            # pin row 0 to exactly 1.0 (it feeds Wo's bias row in the tail).
            for hp in range(2):
                h = 2 * pr + hp
                for qc in range(NQC):
                    sl = slice(qc * QCW, (qc + 1) * QCW)
                    nc.vector.tensor_copy(OT[h][:, sl],
                                          pv[(hp, qc)][0:DH + 1, 0:QCW])
                    rcf = rpool.tile([1, 512], F32, tag="rcf", name="rcf")
                    nc.vector.reciprocal_approx_fast(
                        rcf[:, 0:QCW], OT[h][0:1, sl].bitcast(F32))
                    nc.vector.tensor_copy(OT[h][0:1, sl], rcf[:, 0:QCW])
                    rb = psum_pv.tile([P, 512], F32, tag=f"pv{hp}{qc}",
                                      name=f"rb{hp}{qc}")
                    nc.tensor.matmul(rb[0:DH + 1, 0:QCW], _r(lones[:]),
                                     _r(OT[h][:, sl]), start=True, stop=True)
                    nc.vector.tensor_mul(OT[h][:, sl], OT[h][:, sl].bitcast(F32),
                                         rb[0:DH + 1, 0:QCW])
                    nc.gpsimd.memset(OT[h][0:1, sl].bitcast(F32), 1.0)
        attn.close()
        vstack.close()
        psum_tl = ctx.enter_context(tc.tile_pool(name="psum_tl", bufs=3, space="PSUM"))
        wopool = ctx.enter_context(tc.tile_pool(name="wopool", bufs=1))
        # Wo per head, shifted one row down; row 0 holds bo (head 0) / 0.
        wo_sb = []
        for h in range(H):
            t = wopool.tile([DH + 1, QD], F32R, tag=f"wo{h}", name=f"wo{h}")
            dma(t[1:DH + 1, :], wo_d[h * DH:(h + 1) * DH, :].bitcast(F32R))
            if h == 0:
                dma(t[0:1, :], bo_d[:, :].bitcast(F32R))
            else:
                nc.gpsimd.memset(t[0:1, :].bitcast(F32), 0.0)
            wo_sb.append(t)

        # ---- output projection (everything is already normalized) ----
        for qc in range(NQC):
            for q8 in range(qc * QCW // P, (qc + 1) * QCW // P):
                po = psum_tl.tile([P, 512], F32, tag="po", name="po")
                for h in range(H):
                    nc.tensor.matmul(po[:],
                                     _r(OT[h][:, q8 * P:(q8 + 1) * P]),
                                     _r(wo_sb[h][:, :]),
                                     start=(h == 0), stop=(h == H - 1))
                cp = nc.vector.tensor_copy if q8 % 2 == 0 else nc.scalar.copy
                ost = rpool.tile([P, 512], F32, tag="ost", name="ost")
                cp(ost[:], po[:])
                dma(y_d[q8 * P:(q8 + 1) * P, :], ost[:])

    return nc


def TileCtx(nc):
    return tile.TileContext(nc)


_NC_CACHE = {}


def _get_compiled(NQ=1024, M=2048):
    key = (NQ, M)
    if key not in _NC_CACHE:
        nc = bacc.Bacc("TRN2", target_bir_lowering=False, debug=False)
        build_core_kernel(nc, NQ=NQ, M=M)
        nc.compile()
        _NC_CACHE[key] = nc
    return _NC_CACHE[key]


def _make_in_maps(inputs):
    x = np.ascontiguousarray(np.asarray(inputs["x"], dtype=np.float32))
    context = np.ascontiguousarray(np.asarray(inputs["context"], dtype=np.float32))
    B, N, _ = x.shape
    NQ = N // 2
    common = {
        "wq_in": np.asarray(inputs["Wq"], np.float32),
        "wk_in": np.asarray(inputs["Wk"], np.float32),
        "wv_in": np.asarray(inputs["Wv"], np.float32),
        "wo_in": np.asarray(inputs["Wo"], np.float32),
        "w1_in": np.asarray(inputs["W1"], np.float32),
        "w2_in": np.asarray(inputs["W2"], np.float32).reshape(GH, 1),
        "b1_in": np.asarray(inputs["b1"], np.float32).reshape(GH, 1),
        "b2_in": np.asarray(inputs["b2"], np.float32).reshape(1, 1),
        "bo_in": np.asarray(inputs["bo"], np.float32).reshape(1, QD),
    }
    in_maps = []
    for c in range(8):
        b, qh = c // 2, c % 2
        m = dict(common)
        m["xt_in"] = np.ascontiguousarray(x[b, qh * NQ:(qh + 1) * NQ, :].T)
        m["ctxt_in"] = np.ascontiguousarray(context[b].T)
        in_maps.append(m)
    return in_maps


def kernel(x, context, Wq, Wk, Wv, W1, b1, W2, b2, Wo, bo):
    x = np.ascontiguousarray(np.asarray(x, dtype=np.float32))
    context = np.ascontiguousarray(np.asarray(context, dtype=np.float32))
    B, N, _ = x.shape
    NQ = N // 2
    M = context.shape[1]
    nc = _get_compiled(NQ=NQ, M=M)
    in_maps = _make_in_maps(dict(
        x=x, context=context, Wq=Wq, Wk=Wk, Wv=Wv, W1=W1, b1=b1, W2=W2,
        b2=b2, Wo=Wo, bo=bo))

    res = run_bass_kernel_spmd(nc, in_maps, list(range(8))).results
    out = np.empty((B, N, QD), dtype=np.float32)
    for c in range(8):
        b, qh = c // 2, c % 2
        out[b, qh * NQ:(qh + 1) * NQ, :] = res[c]["y_out"]
    return out


# revision 30
# speedup vs baseline: 1.2138x; 1.0791x over previous
"""Cross-attention Trainium2 kernel (8 NeuronCores, SPMD).

Reference computation (per batch b):
    gate = sigmoid(relu(ctx @ W1 + b1) @ W2 + b2)        # [M, 1]
    ctxg = ctx * gate
    q = x @ Wq; k = ctxg @ Wk; v = ctxg @ Wv             # per head slices of 64
    out = softmax(q k^T / 8) v                           # per head
    y = concat_heads(out) @ Wo + bo                      # [N, 512]

Sharding: 8 cores = 4 batches x 2 query-halves. Each core computes the
FULL output rows for its (batch, 1024-query slice); host gather is pure
concatenation. x and context are pre-transposed on the host so the
kernel never runs a PE transpose.

Performance model this kernel is built around (measured on trn2):
  * The PE's HAM clock gate only reaches 2.4 GHz when the matmul stream
    stays in ONE array-tiling mode; any K<65 matmul switches the array
    to a row-tiled mode and the drain keeps the clock at 1.2 GHz. So
    EVERY matmul here contracts K in [65..128]: zero-padded operands
    make up the difference (KTz pair layout, padded gate weights, a
    65-row ones matmul for broadcasts).
  * The attention inner loop is software-pipelined: per context chunk,
    S-matmuls and the exp() activation for chunk j issue together with
    the PV matmuls of chunk j-1, so the Scalar (ACT) engine — the
    bottleneck at ~2.3us per chunk — never waits on the PE.
  * Softmax normalization (reciprocal + broadcast + multiply) runs
    inside the attention phase on engine slack, reusing the just-freed
    pv PSUM banks; the tail is only the output projection.
  * exp() doubles as PSUM eviction; no max-subtraction (|s| <~ 8).
  * PV lhsT is a 128-col slice [1 | V_h | junk] of Vt: out row 0 =
    softmax denominator, rows 1:65 = V^T E, rows 65:128 junk (never
    read). The ones column is FIRST because DVE reciprocal_approx_fast
    is only valid at partition 0.
"""

import os
import sys
from contextlib import ExitStack

import numpy as np

if "/opt/trn_rl_repo" not in sys.path:
    sys.path.insert(0, "/opt/trn_rl_repo")

import concourse.bass as bass
import concourse.mybir as mybir
import concourse.tile as tile
from concourse import bacc
from concourse.bass_utils import run_bass_kernel_spmd

F32 = mybir.dt.float32
F32R = mybir.dt.float32r
EXPF = mybir.ActivationFunctionType.Exp
RELUF = mybir.ActivationFunctionType.Relu
SIGMF = mybir.ActivationFunctionType.Sigmoid

H = 8          # heads
DH = 64        # dim per head
QD = 512       # query feature dim
CD = 64        # context feature dim
GH = 32        # gate hidden
INNER = H * DH # 512
SCALE = DH ** -0.5
VW = DH + 1    # per-head Vt block width (ones col + V)


def _r(ap):
    return ap.bitcast(F32R)


def build_core_kernel(nc, NQ=1024, M=2048):
    """Emit the per-core kernel. NQ = queries on this core, M = ctx length."""
    P = 128
    NJC = M // P          # ctx 128-chunks
    NG4 = M // 512        # ctx 512-chunks
    NQC = max(NQ // 512, 1)  # query 512-chunks
    QCW = min(512, NQ)    # query chunk width
    NKC = QD // P         # 4 qdim 128-chunks

    xt_d = nc.dram_tensor("xt_in", [QD, NQ], F32, kind="ExternalInput").ap()
    ct_d = nc.dram_tensor("ctxt_in", [CD, M], F32, kind="ExternalInput").ap()
    wq_d = nc.dram_tensor("wq_in", [QD, INNER], F32, kind="ExternalInput").ap()
    wk_d = nc.dram_tensor("wk_in", [CD, INNER], F32, kind="ExternalInput").ap()
    wv_d = nc.dram_tensor("wv_in", [CD, INNER], F32, kind="ExternalInput").ap()
    wo_d = nc.dram_tensor("wo_in", [INNER, QD], F32, kind="ExternalInput").ap()
    w1_d = nc.dram_tensor("w1_in", [CD, GH], F32, kind="ExternalInput").ap()
    w2_d = nc.dram_tensor("w2_in", [GH, 1], F32, kind="ExternalInput").ap()
    b1_d = nc.dram_tensor("b1_in", [GH, 1], F32, kind="ExternalInput").ap()
    b2_d = nc.dram_tensor("b2_in", [1, 1], F32, kind="ExternalInput").ap()
    bo_d = nc.dram_tensor("bo_in", [1, QD], F32, kind="ExternalInput").ap()
    y_d = nc.dram_tensor("y_out", [NQ, QD], F32, kind="ExternalOutput").ap()

    with TileCtx(nc) as tc, ExitStack() as ctx, \
            nc.allow_low_precision(reason="float32r rounding for PE operands"):
        const = ctx.enter_context(tc.tile_pool(name="const", bufs=1))
        persist = ctx.enter_context(tc.tile_pool(name="persist", bufs=1))
        vstack = ExitStack()
        vpool = vstack.enter_context(tc.tile_pool(name="vpool", bufs=1, side="right"))
        xpool = vstack.enter_context(tc.tile_pool(name="xpool", bufs=1, side="right"))
        early = ExitStack()
        cpool = early.enter_context(tc.tile_pool(name="cpool", bufs=1))
        psum_pj = early.enter_context(tc.tile_pool(name="psum_pj", bufs=2, space="PSUM"))

        dma = nc.sync.dma_start
        dma2 = nc.scalar.dma_start

        # ---- constants / weights (zero-padded to K=128 contractions) ----
        ones_f = const.tile([1, P], F32, tag="ones_f", name="ones_f")
        nc.vector.memset(ones_f[:], 1.0)
        # Lones65: row 0 = ones, rows 1:65 = 0. As lhsT with a [65, N] rhs it
        # broadcasts the rhs's row 0 to 65 output rows while contracting
        # K=65 (which rounds to the 128x128 PE mode — no mode switch).
        lones = const.tile([DH + 1, DH + 1], F32R, tag="lones", name="lones")
        nc.vector.memset(lones[:].bitcast(F32), 0.0)
        nc.vector.memset(lones[0:1, :].bitcast(F32), 1.0)

        wk_sb = const.tile([P, INNER], F32R, tag="wk", name="wk")
        dma(wk_sb[0:CD, :], wk_d[:, :].bitcast(F32R))
        nc.gpsimd.memset(wk_sb[CD:P, :].bitcast(F32), 0.0)
        wv_sb = const.tile([P, INNER], F32R, tag="wv", name="wv")
        dma(wv_sb[0:CD, :], wv_d[:, :].bitcast(F32R))
        nc.gpsimd.memset(wv_sb[CD:P, :].bitcast(F32), 0.0)
        w1_sb = const.tile([P, GH], F32R, tag="w1", name="w1")
        dma(w1_sb[0:CD, :], w1_d[:, :].bitcast(F32R))
        nc.vector.memset(w1_sb[CD:P, :].bitcast(F32), 0.0)
        w2_sb = const.tile([P, 1], F32R, tag="w2", name="w2")
        dma(w2_sb[0:GH, :], w2_d[:, :].bitcast(F32R))
        nc.vector.memset(w2_sb[GH:2 * GH, :].bitcast(F32), 0.0)
        nc.vector.memset(w2_sb[2 * GH:P, :].bitcast(F32), 0.0)
        b1_sb = const.tile([GH, 1], F32, tag="b1", name="b1")
        dma(b1_sb[:], b1_d[:, :])
        b2_sb = const.tile([1, 1], F32, tag="b2", name="b2")
        dma(b2_sb[:], b2_d[:, :])

        # x^T and Wq ride the Activation-engine HWDGE queue so they overlap
        # the ctx-chain DMAs on the SP queue.
        wq_sb = [xpool.tile([P, INNER], F32R, tag=f"wq{k}", name=f"wq{k}") for k in range(NKC)]
        for k in range(NKC):
            dma2(wq_sb[k][:], wq_d[k * P:(k + 1) * P, :].bitcast(F32R))
        xT = [xpool.tile([P, NQ], F32R, tag=f"xT{k}", name=f"xT{k}") for k in range(NKC)]
        for k in range(NKC):
            dma2(xT[k][:], xt_d[k * P:(k + 1) * P, :].bitcast(F32R))

        # ---- context (pre-transposed on host), bottom rows zeroed ----
        ctxT = cpool.tile([P, M], F32R, tag="ctxT", name="ctxT")
        dma(ctxT[0:CD, :], ct_d[:, :].bitcast(F32R))
        nc.gpsimd.memset(ctxT[CD:P, :].bitcast(F32), 0.0)

        # ---- gate + gated context: ctxgT [128, M] (bottom zero) ----
        ctxgT = cpool.tile([P, M], F32R, tag="ctxgT", name="ctxgT")
        nc.gpsimd.memset(ctxgT[CD:P, :].bitcast(F32), 0.0)
        h1z = cpool.tile([P, 512], F32R, tag="h1z", name="h1z")
        nc.vector.memset(h1z[GH:2 * GH, :].bitcast(F32), 0.0)
        nc.vector.memset(h1z[2 * GH:P, :].bitcast(F32), 0.0)
        gtz = cpool.tile([DH + 1, 512], F32R, tag="gtz", name="gtz")
        nc.vector.memset(gtz[:].bitcast(F32), 0.0)
        for g in range(NG4):
            sl = slice(g * 512, (g + 1) * 512)
            pp = psum_pj.tile([P, 512], F32, tag="pj", name="pj")
            nc.tensor.matmul(pp[0:GH, :], _r(w1_sb[:]), _r(ctxT[:, sl]),
                             start=True, stop=True)
            nc.scalar.activation(h1z[0:GH, :], pp[0:GH, :], RELUF, bias=b1_sb[:])
            pp2 = psum_pj.tile([P, 512], F32, tag="pj", name="pj")
            nc.tensor.matmul(pp2[0:1, :], _r(w2_sb[:]), _r(h1z[:]),
                             start=True, stop=True)
            nc.scalar.activation(gtz[0:1, :], pp2[0:1, :], SIGMF, bias=b2_sb[:])
            ppb = psum_pj.tile([P, 512], F32, tag="pj", name="pj")
            nc.tensor.matmul(ppb[0:CD, :], _r(lones[:, 0:CD]), _r(gtz[:]),
                             start=True, stop=True)
            nc.vector.tensor_mul(ctxgT[0:CD, sl], ctxT[0:CD, sl], ppb[0:CD, :])

        # ---- K^T, zero-padded pair layout: KTz[h] [128, M] ----
        # even head: data rows 0:64, zeros 64:128; odd head: the reverse —
        # matching QT's pair layout so S contracts K=128 against zeros.
        KTz = [persist.tile([P, M], F32R, tag=f"ktz{h}", name=f"ktz{h}") for h in range(H)]
        for h in range(H):
            z = slice(DH, P) if h % 2 == 0 else slice(0, DH)
            nc.gpsimd.memset(KTz[h][z, :].bitcast(F32), 0.0)
        for pr in range(H // 2):
            for g in range(NG4):
                sl = slice(g * 512, (g + 1) * 512)
                pp = psum_pj.tile([P, 512], F32, tag="pj", name="pj")
                nc.tensor.matmul(pp[:], _r(wk_sb[:, pr * P:(pr + 1) * P]),
                                 _r(ctxgT[:, sl]), start=True, stop=True)
                nc.vector.tensor_copy(KTz[2 * pr][0:DH, sl], pp[0:DH, :])
                nc.scalar.copy(KTz[2 * pr + 1][DH:P, sl], pp[DH:P, :])

        # ---- Q^T pair 0 first so attention can start early ----
        QT = [persist.tile([P, NQ], F32R, tag=f"qt{pr}", name=f"qt{pr}") for pr in range(H // 2)]

        def gen_qt(pr):
            for qc in range(NQC):
                sl = slice(qc * QCW, (qc + 1) * QCW)
                if pr == 0:
                    pp = psum_pj.tile([P, 512], F32, tag="pj", name="pj")
                else:
                    pp = psum_st.tile([P, NQC * 512], F32, tag=f"st{qc}",
                                      name="qtp")
                for k in range(NKC):
                    nc.tensor.matmul(pp[:, 0:QCW],
                                     _r(wq_sb[k][:, pr * P:(pr + 1) * P]),
                                     _r(xT[k][:, sl]),
                                     start=(k == 0), stop=(k == NKC - 1))
                # ACT is saturated with exp() during attention — only use it
                # for QT eviction in the early phase (pr == 0).
                cp = nc.scalar.copy if (pr == 0 and qc % 2 == 1) \
                    else nc.vector.tensor_copy
                cp(QT[pr][:, sl], pp[:, 0:QCW])

        gen_qt(0)

        # ---- V natural, interleaved [1 | V_h] blocks of 65 + 63-col pad so
        # every head has a 128-col stationary slice: Vt[t] [128, 583] ----
        VTW = H * VW + (P - VW)
        Vt = [vpool.tile([P, VTW], F32R, tag=f"v{t}", name=f"v{t}") for t in range(NJC)]
        for t in range(NJC):
            nc.gpsimd.memset(Vt[t][:].bitcast(F32), 0.0)
            vv = Vt[t][:, 0:H * VW].rearrange("p (h c) -> p h c", c=VW)
            nc.gpsimd.memset(vv[:, :, 0:1].bitcast(F32), 1.0)
            pp = psum_pj.tile([P, 512], F32, tag="pj", name="pj")
            nc.tensor.matmul(pp[:], _r(ctxgT[:, t * P:(t + 1) * P]), _r(wv_sb[:]),
                             start=True, stop=True)
            cp = nc.vector.tensor_copy if t % 2 == 0 else nc.scalar.copy
            cp(vv[:, :, 1:VW], pp[:].rearrange("p (h c) -> p h c", c=DH))

        early.close()
        attn = ExitStack()
        psum_st = attn.enter_context(tc.tile_pool(name="psum_st", bufs=1, space="PSUM"))
        psum_pv = attn.enter_context(tc.tile_pool(name="psum_pv", bufs=1, space="PSUM"))
        epool = ctx.enter_context(tc.tile_pool(name="epool", bufs=2))
        rpool = ctx.enter_context(tc.tile_pool(name="rpool", bufs=2))

        # ---- attention, software-pipelined per context chunk ----
        # Per jc: S(h0), EXP(h0), PV(h0, jc-1), S(h1), EXP(h1), PV(h1, jc-1)
        # so the ACT engine (the ~2.3us/jc bottleneck) runs back to back.
        OT = [persist.tile([DH + 1, NQ], F32R, tag=f"ot{h}", name=f"ot{h}") for h in range(H)]
        def normalize(pr):
            # Broadcast 1/denom (row 0, written a pair ago) over 65 rows via
            # the ones-row matmul, multiply, pin row 0 to exactly 1.0.
            for hp in range(2):
                h = 2 * pr + hp
                for qc in range(NQC):
                    sl = slice(qc * QCW, (qc + 1) * QCW)
                    rb = psum_pv.tile([P, 512], F32, tag=f"pv{hp}{qc}",
                                      name=f"rb{hp}{qc}")
                    nc.tensor.matmul(rb[0:DH + 1, 0:QCW], _r(lones[:]),
                                     _r(OT[h][:, sl]), start=True, stop=True)
                    nc.vector.tensor_mul(OT[h][:, sl], OT[h][:, sl].bitcast(F32),
                                         rb[0:DH + 1, 0:QCW])
                    nc.gpsimd.memset(OT[h][0:1, sl].bitcast(F32), 1.0)

        for pr in range(H // 2):
            pv = {(hp, qc): psum_pv.tile([P, 512], F32,
                                         tag=f"pv{hp}{qc}", name=f"pv{hp}{qc}")
                  for hp in range(2) for qc in range(NQC)}
            et_prev = [None, None]
            for jc in range(NJC):
                et_cur = [None, None]
                for hp in range(2):
                    st = psum_st.tile([P, NQC * 512], F32, tag=f"st{hp}",
                                      name=f"st{hp}")
                    for qc in range(NQC):
                        nc.tensor.matmul(
                            st[:, qc * 512:qc * 512 + QCW],
                            _r(KTz[2 * pr + hp][:, jc * P:(jc + 1) * P]),
                            _r(QT[pr][:, qc * QCW:(qc + 1) * QCW]),
                            start=True, stop=True)
                    et = epool.tile([P, NQC * 512], F32R, tag=f"e{hp}",
                                    name=f"et{hp}")
                    nc.scalar.activation(et[:], st[:], EXPF, scale=SCALE)
                    et_cur[hp] = et
                    if jc > 0:
                        h = 2 * pr + hp
                        for qc in range(NQC):
                            nc.tensor.matmul(
                                pv[(hp, qc)][:, 0:QCW],
                                _r(Vt[jc - 1][:, h * VW:h * VW + P]),
                                _r(et_prev[hp][:, qc * 512:qc * 512 + QCW]),
                                start=(jc == 1), stop=False)
                et_prev = et_cur
            for hp in range(2):
                h = 2 * pr + hp
                for qc in range(NQC):
                    nc.tensor.matmul(
                        pv[(hp, qc)][:, 0:QCW],
                        _r(Vt[NJC - 1][:, h * VW:h * VW + P]),
                        _r(et_prev[hp][:, qc * 512:qc * 512 + QCW]),
                        start=False, stop=True)
            # Next pair's QT projection first: its eviction copies lead the
            # DVE queue so the next pair's S matmuls wait only on them.
            if pr + 1 < H // 2:
                gen_qt(pr + 1)
            # Evict rows 0:65 (row 0 = denominator); replace the denominator
            # with its reciprocal on DVE slack.
            for hp in range(2):
                h = 2 * pr + hp
                for qc in range(NQC):
                    sl = slice(qc * QCW, (qc + 1) * QCW)
                    nc.vector.tensor_copy(OT[h][:, sl],
                                          pv[(hp, qc)][0:DH + 1, 0:QCW])
                    rcf = rpool.tile([1, 512], F32, tag="rcf", name="rcf")
                    nc.vector.reciprocal_approx_fast(
                        rcf[:, 0:QCW], OT[h][0:1, sl].bitcast(F32))
                    nc.vector.tensor_copy(OT[h][0:1, sl], rcf[:, 0:QCW])
            # Normalize the PREVIOUS pair: its reciprocals are a full pair
            # old, so these matmuls never make the PE wait on the DVE chain.
            if pr > 0:
                normalize(pr - 1)
        normalize(H // 2 - 1)
        attn.close()
        vstack.close()
        psum_tl = ctx.enter_context(tc.tile_pool(name="psum_tl", bufs=3, space="PSUM"))
        wopool = ctx.enter_context(tc.tile_pool(name="wopool", bufs=1))
        # Wo per head, shifted one row down; row 0 holds bo (head 0) / 0.
        wo_sb = []
        for h in range(H):
            t = wopool.tile([DH + 1, QD], F32R, tag=f"wo{h}", name=f"wo{h}")
            dma(t[1:DH + 1, :], wo_d[h * DH:(h + 1) * DH, :].bitcast(F32R))
            if h == 0:
                dma(t[0:1, :], bo_d[:, :].bitcast(F32R))
            else:
                nc.gpsimd.memset(t[0:1, :].bitcast(F32), 0.0)
            wo_sb.append(t)

        # ---- output projection (everything is already normalized) ----
        for qc in range(NQC):
            for q8 in range(qc * QCW // P, (qc + 1) * QCW // P):
                po = psum_tl.tile([P, 512], F32, tag="po", name="po")
                for h in range(H):
                    nc.tensor.matmul(po[:],
                                     _r(OT[h][:, q8 * P:(q8 + 1) * P]),
                                     _r(wo_sb[h][:, :]),
                                     start=(h == 0), stop=(h == H - 1))
                cp = nc.vector.tensor_copy if q8 % 2 == 0 else nc.scalar.copy
                ost = rpool.tile([P, 512], F32, tag="ost", name="ost")
                cp(ost[:], po[:])
                dma(y_d[q8 * P:(q8 + 1) * P, :], ost[:])

    return nc


def TileCtx(nc):
    return tile.TileContext(nc)


_NC_CACHE = {}


def _get_compiled(NQ=1024, M=2048):
    key = (NQ, M)
    if key not in _NC_CACHE:
        nc = bacc.Bacc("TRN2", target_bir_lowering=False, debug=False)
        build_core_kernel(nc, NQ=NQ, M=M)
        nc.compile()
        _NC_CACHE[key] = nc
    return _NC_CACHE[key]


def _make_in_maps(inputs):
    x = np.ascontiguousarray(np.asarray(inputs["x"], dtype=np.float32))
    context = np.ascontiguousarray(np.asarray(inputs["context"], dtype=np.float32))
    B, N, _ = x.shape
    NQ = N // 2
    common = {
        "wq_in": np.asarray(inputs["Wq"], np.float32),
        "wk_in": np.asarray(inputs["Wk"], np.float32),
        "wv_in": np.asarray(inputs["Wv"], np.float32),
        "wo_in": np.asarray(inputs["Wo"], np.float32),
        "w1_in": np.asarray(inputs["W1"], np.float32),
        "w2_in": np.asarray(inputs["W2"], np.float32).reshape(GH, 1),
        "b1_in": np.asarray(inputs["b1"], np.float32).reshape(GH, 1),
        "b2_in": np.asarray(inputs["b2"], np.float32).reshape(1, 1),
        "bo_in": np.asarray(inputs["bo"], np.float32).reshape(1, QD),
    }
    in_maps = []
    for c in range(8):
        b, qh = c // 2, c % 2
        m = dict(common)
        m["xt_in"] = np.ascontiguousarray(x[b, qh * NQ:(qh + 1) * NQ, :].T)
        m["ctxt_in"] = np.ascontiguousarray(context[b].T)
        in_maps.append(m)
    return in_maps


def kernel(x, context, Wq, Wk, Wv, W1, b1, W2, b2, Wo, bo):
    x = np.ascontiguousarray(np.asarray(x, dtype=np.float32))
    context = np.ascontiguousarray(np.asarray(context, dtype=np.float32))
    B, N, _ = x.shape
    NQ = N // 2
    M = context.shape[1]
    nc = _get_compiled(NQ=NQ, M=M)
    in_maps = _make_in_maps(dict(
        x=x, context=context, Wq=Wq, Wk=Wk, Wv=Wv, W1=W1, b1=b1, W2=W2,
        b2=b2, Wo=Wo, bo=bo))

    res = run_bass_kernel_spmd(nc, in_maps, list(range(8))).results
    out = np.empty((B, N, QD), dtype=np.float32)
    for c in range(8):
        b, qh = c // 2, c % 2
        out[b, qh * NQ:(qh + 1) * NQ, :] = res[c]["y_out"]
    return out
